# revision 2
# baseline (speedup 1.0000x reference)
"""Trainium2 Bass kernel for nn_AdjAttenAgger (masked cross-attention
aggregation), running SPMD on 8 NeuronCores.

Math (row-sharded 8 ways over NQ=16384):
  Q = g @ Wq.T + bq                      [R, 256]
  K = sub @ Wk.T + bk                    [4096, 256]
  S = (Q @ K.T) / sqrt(256)              [R, 4096]
  attn = softmax(S masked by mask)       row-wise
  out = attn @ (diag(w) @ sub)           [R, 256]

Implementation notes:
- Host-side prep (dtype/layout only, no model math): the 0/1 int32 mask is
  cast to fp8 (exact) and g/sub/Wq/Wk to bf16 before staging, so the
  device reads 8MB of mask instead of 32MB and skips all on-device
  f32->bf16 / i32->fp8 casts (the i32->fp8 DVE cast alone was ~5us/macro).
- Scores are built TRANSPOSED (s^T [nk, q]) in PSUM so that exp() writes P^T
  directly to SBUF and the second gemm (contraction over nk) needs no
  transposes of P.
- The mask is folded in additively BEFORE exp: PE transpose-matmuls of the
  fp8 mask against a scaled identity (640*I) accumulate 640*mask^T into the
  scores PSUM; with exp(x/16) that is exp(s + 40*mask): kept entries carry a
  uniform e^40 factor that cancels in the softmax normalization, masked
  entries are suppressed by ~e^-40 (~4e-18, far below the accuracy target).
  This avoids a separate masked-max/`-inf` pass entirely; |s| is small
  (~N(0,1) scores) so the unstabilized exp cannot overflow.
- The denominator rides along as a 257th "ones" column of V: one extra PSUM
  column per output tile, then a reciprocal multiply on the way out.
- All matmuls run in bf16 (inputs are well-scaled; measured output rel err
  ~4e-3); the mask is fp8 (0/1 exact) so its LDWEIGHTS stream is 4x faster.
- DMA queues are specialized: the mask (dominant input) and outputs ride
  the sync HWDGE queue; substruct/global embeddings keep out of its way
  (g on the SWDGE queue); the small projection weights use the scalar
  HWDGE queue so their completion is never round-robined behind bulk
  packets. substruct_weight is loaded contiguously [32,128] and
  PE-transposed (the direct strided gather costs ~8us of queue
  head-of-line time).
- Mask tiles are prefetched two macros ahead, and both macro-0 tiles' first
  halves land before their second halves, so the PE never stalls on DMA.
- Each macro's output gemm is software-pipelined into the next macro's PE
  stream; the last macro's own output gemm interleaves into the second half
  of its own score loop (each piece only reads pT columns written by a
  program-order-earlier exp).
- A PE warm-up matmul with no DMA deps issues first thing so the HAM
  clock-gate is less likely to hold the PE at half clock across the
  DMA-dominated ramp-in.
"""
from contextlib import ExitStack

import ml_dtypes
import numpy as np

import concourse.bass as bass
import concourse.tile as tile
from concourse import bacc, masks, mybir
from concourse.bass_utils import run_bass_kernel_spmd

F32 = mybir.dt.float32
BF16 = mybir.dt.bfloat16
FP8 = mybir.dt.float8e4
AF = mybir.ActivationFunctionType
OP = mybir.AluOpType

NQ, NK = 16384, 4096
QDIM, KDIM, MID = 512, 256, 256
N_CORES = 8
R = NQ // N_CORES            # 2048 rows per core
QMAC = 256                   # q-rows per macro block
BIG = 40.0                   # mask offset after the /16 exp scale

NP_BF16 = ml_dtypes.bfloat16
NP_FP8 = ml_dtypes.float8_e4m3


def _make_scaled_identity(nc, ident_ap, fill):
    nc.gpsimd.memset(ident_ap, 0.0)
    nc.gpsimd.affine_select(
        out=ident_ap, in_=ident_ap, compare_op=OP.not_equal, fill=fill,
        base=0, pattern=[[-1, ident_ap.shape[0]]], channel_multiplier=1,
    )


def _build(loop_n=1):
    NMAC = R // QMAC
    TPM = QMAC // 128
    NKC = NK // 128
    BIG16 = BIG * 16.0
    nc = bacc.Bacc("TRN2", target_bir_lowering=False, debug=False,
                   num_devices=N_CORES)

    g_d = nc.dram_tensor("global_embeddings", [R, QDIM], BF16, kind="ExternalInput").ap()
    sub_d = nc.dram_tensor("substruct_embeddings", [NK, KDIM], BF16, kind="ExternalInput").ap()
    w_d = nc.dram_tensor("substruct_weight", [NK], F32, kind="ExternalInput").ap()
    mask_d = nc.dram_tensor("mask", [R, NK], FP8, kind="ExternalInput").ap()
    wq_d = nc.dram_tensor("Wq", [MID, QDIM], BF16, kind="ExternalInput").ap()
    bq_d = nc.dram_tensor("bq", [MID], F32, kind="ExternalInput").ap()
    wk_d = nc.dram_tensor("Wk", [MID, KDIM], BF16, kind="ExternalInput").ap()
    bk_d = nc.dram_tensor("bk", [MID], F32, kind="ExternalInput").ap()
    out_d = nc.dram_tensor("out", [R, KDIM], F32, kind="ExternalOutput").ap()

    with tile.TileContext(nc) as tc, ExitStack() as ctx:
        const = ctx.enter_context(tc.tile_pool(name="const", bufs=1))
        kv = ctx.enter_context(tc.tile_pool(name="kv", bufs=1))
        io = ctx.enter_context(tc.tile_pool(name="io", bufs=3))
        iombf = ctx.enter_context(tc.tile_pool(name="iombf", bufs=3 * TPM))
        work = ctx.enter_context(tc.tile_pool(name="work", bufs=2))
        prol = ctx.enter_context(tc.tile_pool(name="prol", bufs=1))
        prol4 = ctx.enter_context(tc.tile_pool(name="prol4", bufs=4))
        ps_s = ctx.enter_context(tc.tile_pool(name="ps_s", bufs=3, space="PSUM"))
        ps_sm = ctx.enter_context(tc.tile_pool(name="ps_sm", bufs=2, space="PSUM"))
        ps_go = ctx.enter_context(tc.tile_pool(name="ps_go", bufs=3, space="PSUM"))

        loop_cm = tc.For_i(0, loop_n) if loop_n != 1 else None

        def body():
            ident = const.tile([128, 128], BF16, tag="ident")
            masks.make_identity(nc, ident[:])
            identB = const.tile([128, 128], BF16, tag="identB")
            _make_scaled_identity(nc, identB[:], BIG16)

            def warm_burst(n):
                # HAM warmers: bursts of PE matmuls with no DMA deps. The
                # clock-gate only releases (1.2->2.4 GHz) after ~3.4us of
                # SUSTAINED PE activity, and re-throttles after ~3.4us idle;
                # the DMA-bound ramp-in is too choppy on its own, so without
                # these the first ~40us runs at half clock. Each burst gets a
                # fresh pool tile: re-targeting an old one would alias a
                # rotated bank under a live tile (PSUM collision).
                wt = ps_sm.tile([128, 128], F32, tag="ps_small")
                for _ in range(n):
                    nc.tensor.matmul(wt[:], ident[:], ident[:])

            # fire the un-throttle window as early as possible
            warm_burst(48)

            wqT = const.tile([128, 4, 2, 128], BF16, tag="wqT")
            wkT = const.tile([128, 2, 2, 128], BF16, tag="wkT")
            v_sb = kv.tile([128, NKC, KDIM + 1], BF16, tag="v_sb")
            kT = kv.tile([128, 2, NK], BF16, tag="kT")

            mask_bfs = {}

            def load_mask(tile_idx):
                # half-tile loads: PE mask matmuls can start as soon as the
                # first half lands
                row = tile_idx * 128
                mbf = iombf.tile([128, NK], FP8, tag="mask_bf")
                for h in range(2):
                    nc.sync.dma_start(
                        mbf[:, bass.ts(h, NK // 2)],
                        mask_d[row : row + 128, bass.ts(h, NK // 2)],
                    )
                mask_bfs[tile_idx] = mbf

            # ------------- prologue: DMA issue + weight prep -------------
            # small weight tensors on the scalar HWDGE queue: their completion
            # is never round-robined against bulk mask/sub packets
            wq_bf = prol.tile([128, 2, QDIM], BF16, tag="wq_bf")
            nc.scalar.dma_start(wq_bf[:], wq_d.rearrange("(m p) d -> p m d", p=128))
            bqT = const.tile([128, 2], F32, tag="bqT")
            nc.scalar.dma_start(bqT[:], bq_d.rearrange("(m p) -> p m", p=128))
            bkT = const.tile([128, 2], F32, tag="bkT")
            nc.scalar.dma_start(bkT[:], bk_d.rearrange("(m p) -> p m", p=128))
            # substruct_weight: a direct "(c p) -> p c" DMA would be a
            # 4-byte-element stride-512B gather (~8us of queue time); load it
            # contiguously [32,128] and transpose on the PE instead
            w_r = prol.tile([32, 128], F32, tag="w_r")
            nc.scalar.dma_start(w_r[:], w_d.rearrange("(c p) -> c p", p=128))
            wk_bf = prol.tile([128, 2, KDIM], BF16, tag="wk_bf")
            nc.scalar.dma_start(wk_bf[:], wk_d.rearrange("(m p) d -> p m d", p=128))

            # g tiles for macro 0 prefetch on the gpsimd (SWDGE) queue
            g_pref = {}
            for t in range(TPM):
                g_bf = io.tile([128, QDIM], BF16, tag="g_bf")
                nc.gpsimd.dma_start(g_bf[:], g_d[t * 128 : t * 128 + 128, :])
                g_pref[t] = g_bf

            w_rb = prol.tile([32, 128], BF16, tag="w_rb")
            nc.vector.tensor_copy(w_rb[:], w_r[:])
            w_sb = const.tile([128, NKC], F32, tag="w_sb")
            pw = ps_sm.tile([128, NKC], F32, tag="ps_small")
            nc.tensor.matmul(pw[:], w_rb[:], ident[0:32, 0:32])
            nc.vector.tensor_copy(w_sb[:], pw[:])
            for m in range(2):
                pt = ps_sm.tile([128, 4, 128], F32, tag="ps_small")
                for qi in range(4):
                    nc.tensor.matmul(
                        pt[:, qi, :], wq_bf[:, m, bass.ts(qi, 128)], ident[:]
                    )
                nc.vector.tensor_copy(wqT[:, :, m, :], pt[:])

            for m in range(2):
                pt = ps_sm.tile([128, 2, 128], F32, tag="ps_small")
                for kc in range(2):
                    nc.tensor.matmul(
                        pt[:, kc, :], wk_bf[:, m, bass.ts(kc, 128)], ident[:]
                    )
                nc.vector.tensor_copy(wkT[:, :, m, :], pt[:])

            # substruct embeddings: bf16 loads on the sync queue AHEAD of
            # the masks (strict FIFO gives them priority — K^T gates macro 0)
            sub_r = sub_d.rearrange("(c p) d -> p c d", p=128)
            sub_tiles = {}
            for c4 in range(NKC // 4):
                sub_b4 = prol4.tile([128, 4, KDIM], BF16, tag="sub_b4")
                nc.sync.dma_start(sub_b4[:], sub_r[:, bass.ts(c4, 4), :])
                sub_tiles[c4] = sub_b4

            # prologue masks behind sub on the sync queue: both macro-0 tiles'
            # first halves, then their second halves, then macro-1's tiles
            pmbf = {}
            for t in range(TPM):
                mbf = iombf.tile([128, NK], FP8, tag="mask_bf")
                pmbf[t] = mbf
            for h in range(2):
                for t in range(TPM):
                    nc.sync.dma_start(
                        pmbf[t][:, bass.ts(h, NK // 2)],
                        mask_d[t * 128 : t * 128 + 128, bass.ts(h, NK // 2)],
                    )
            mask_bfs.update(pmbf)
            for t in range(TPM, 2 * TPM):
                load_mask(t)

            subT = prol.tile([128, 2, NKC, 128], BF16, tag="subT")
            nc.gpsimd.memset(v_sb[:, :, KDIM : KDIM + 1], 1.0)
            warm_burst(24)

            # all scales+transposes first (dense PE stream), then the whole K^T
            # build — per-group interleaving measured slower (chops the PE
            # stream; the HAM clock-gate stays cold longer). Mini warm-bursts
            # bridge the sub-DMA arrival gaps (measured ~4us each, > the 3.4us
            # re-throttle window).
            for gr in range(NKC // 4):
                warm_burst(12)
                sub_b4 = sub_tiles[gr]
                for c in range(4):
                    nc.vector.tensor_scalar(
                        v_sb[:, gr * 4 + c, 0:KDIM], sub_b4[:, c, :],
                        w_sb[:, gr * 4 + c : gr * 4 + c + 1], None, OP.mult
                    )
                for kc in range(2):
                    pt = ps_sm.tile([128, 4, 128], F32, tag="ps_small")
                    for j in range(4):
                        nc.tensor.matmul(
                            pt[:, j, :], sub_b4[:, j, bass.ts(kc, 128)], ident[:]
                        )
                    if kc == 0:
                        nc.scalar.copy(subT[:, kc, bass.ts(gr, 4), :], pt[:])
                    else:
                        nc.vector.tensor_copy(subT[:, kc, bass.ts(gr, 4), :], pt[:])
            for m in range(2):
                for gr in range(NKC // 4):
                    pk = ps_sm.tile([128, 512], F32, tag="ps_small")
                    for kc in range(2):
                        nc.tensor.matmul(
                            pk[:],
                            wkT[:, kc, m, :],
                            subT[:, kc, bass.ts(gr, 4), :].rearrange("p a b -> p (a b)"),
                            start=(kc == 0),
                            stop=(kc == 1),
                        )
                    nc.scalar.activation(
                        kT[:, m, bass.ts(gr, 512)], pk[:], AF.Identity,
                        bias=bkT[:, m : m + 1], scale=1.0,
                    )

            # ------------- main loop over q macros -------------
            pT_prev = None
            for mac in range(NMAC):
                gT = work.tile([128, 4, QMAC], BF16, tag="gT")
                for t in range(TPM):
                    if mac == 0:
                        g_bf = g_pref[t]
                    else:
                        row = mac * QMAC + t * 128
                        g_bf = io.tile([128, QDIM], BF16, tag="g_bf")
                        nc.gpsimd.dma_start(g_bf[:], g_d[row : row + 128, :])
                    pg = ps_go.tile([128, 4, 128], F32, tag="ps_go")
                    for qi in range(4):
                        nc.tensor.matmul(pg[:, qi, :], g_bf[:, bass.ts(qi, 128)], ident[:])
                    nc.vector.tensor_copy(gT[:, :, bass.ts(t, 128)], pg[:])

                qT = work.tile([128, 2, QMAC], BF16, tag="qT")
                for m in range(2):
                    pq = ps_sm.tile([128, QMAC], F32, tag="ps_small")
                    for qi in range(4):
                        nc.tensor.matmul(
                            pq[:], wqT[:, qi, m, :], gT[:, qi, :],
                            start=(qi == 0), stop=(qi == 3),
                        )
                    nc.vector.tensor_scalar(
                        qT[:, m, :], pq[:], bqT[:, m : m + 1], None, OP.add
                    )

                if mac + 2 < NMAC:
                    for t in range(TPM):
                        load_mask((mac + 2) * TPM + t)

                pT = work.tile([128, NKC, QMAC], BF16, tag="pT")
                bank = 512  # fp32 elems per PSUM bank per partition

                def g2_piece(pmac, pT_src, t, c, g2_state):
                    # one chunk's worth of an output gemm, interleaved into this
                    # macro's PE stream to avoid an end-of-macro gemm2 barrier
                    po = g2_state.get(t)
                    if po is None:
                        po = ps_go.tile([128, KDIM + 1], F32, tag="ps_go")
                        g2_state[t] = po
                    nc.tensor.matmul(
                        po[:], pT_src[:, c, bass.ts(t, 128)], v_sb[:, c, :],
                        start=(c == 0), stop=(c == NKC - 1),
                    )
                    if c == NKC - 1:
                        rec = io.tile([128, 1], F32, tag="rec")
                        nc.vector.reciprocal(rec[:], po[:, KDIM : KDIM + 1])
                        o_sb = io.tile([128, KDIM], F32, tag="o_sb")
                        nc.vector.tensor_scalar(
                            o_sb[:], po[:, 0:KDIM], rec[:, 0:1], None, OP.mult
                        )
                        row = pmac * QMAC + t * 128
                        nc.sync.dma_start(out_d[row : row + 128, :], o_sb[:])

                g2_state = {}
                g2_state_own = {}
                NCP = NKC // 2
                per = (TPM * NKC + NCP - 1) // NCP
                per_own = (TPM * NKC) // (NCP - NCP // 2)
                for cp in range(NCP):
                    ps = ps_s.tile([128, 2, QMAC], F32, tag="ps_sc")
                    for j in range(2):
                        c = cp * 2 + j
                        for t in range(TPM):
                            nc.tensor.matmul(
                                ps[:, j, bass.ts(t, 128)],
                                mask_bfs[mac * TPM + t][:, bass.ts(c, 128)],
                                identB[:],
                                start=((j * QMAC + t * 128) % bank == 0), stop=False,
                            )
                    for j in range(2):
                        c = cp * 2 + j
                        for m in range(2):
                            nc.tensor.matmul(
                                ps[:, j, :],
                                kT[:, m, bass.ts(c, 128)],
                                qT[:, m, :],
                                start=False,
                                stop=(m == 1 and ((j + 1) * QMAC) % bank == 0),
                            )
                    nc.scalar.activation(
                        pT[:, bass.ts(cp, 2), :], ps[:], AF.Exp, scale=0.0625
                    )
                    if mac > 0:
                        for k in range(cp * per, min((cp + 1) * per, TPM * NKC)):
                            t, c = divmod(k, NKC)
                            g2_piece(mac - 1, pT_prev, t, c, g2_state)
                    if mac == NMAC - 1 and cp >= NCP // 2:
                        # the last macro's own gemm2: both chains advance in
                        # parallel, per_own//TPM chunks per cp each, so every
                        # piece only reads pT columns already produced by a
                        # program-order-earlier exp: c <= 2*cp+1
                        for t in range(TPM):
                            for c in range((cp - NCP // 2) * per_own // TPM,
                                           (cp - NCP // 2 + 1) * per_own // TPM):
                                g2_piece(mac, pT, t, c, g2_state_own)

                pT_prev = pT

        if loop_cm is not None:
            with loop_cm:
                body()
        else:
            body()

    nc.compile()
    return nc


def prep_in_maps(inputs):
    """Host-side dtype casts + row-sharding (no model math): returns the
    8 per-core input dicts matching _build()'s declared dram tensors."""
    g = np.asarray(inputs["global_embeddings"]).astype(NP_BF16)
    sub = np.asarray(inputs["substruct_embeddings"]).astype(NP_BF16)
    w = np.ascontiguousarray(np.asarray(inputs["substruct_weight"], dtype=np.float32))
    mask = np.asarray(inputs["mask"]).astype(NP_FP8)
    Wq = np.asarray(inputs["Wq"]).astype(NP_BF16)
    bq = np.ascontiguousarray(np.asarray(inputs["bq"], dtype=np.float32))
    Wk = np.asarray(inputs["Wk"]).astype(NP_BF16)
    bk = np.ascontiguousarray(np.asarray(inputs["bk"], dtype=np.float32))

    in_maps = []
    for i in range(N_CORES):
        sl = slice(i * R, (i + 1) * R)
        in_maps.append({
            "global_embeddings": np.ascontiguousarray(g[sl]),
            "mask": np.ascontiguousarray(mask[sl]),
            "substruct_embeddings": sub, "substruct_weight": w,
            "Wq": Wq, "bq": bq, "Wk": Wk, "bk": bk,
        })
    return in_maps


_CACHE = {}


def kernel(**inputs) -> np.ndarray:
    """Full-input entry point: shards NQ across 8 NeuronCores, runs the Bass
    kernel, and gathers the full [16384, 256] float32 output."""
    if "nc" not in _CACHE:
        _CACHE["nc"] = _build()
    nc = _CACHE["nc"]

    in_maps = prep_in_maps(inputs)
    res = run_bass_kernel_spmd(nc, in_maps, list(range(N_CORES))).results
    return np.concatenate([res[i]["out"] for i in range(N_CORES)], axis=0)


# revision 5
# speedup vs baseline: 1.2858x; 1.2858x over previous
"""Trainium2 Bass kernel for nn_AdjAttenAgger (masked cross-attention
aggregation), running SPMD on 8 NeuronCores.

Math (row-sharded 8 ways over NQ=16384):
  Q = g @ Wq.T + bq                      [R, 256]
  K = sub @ Wk.T + bk                    [4096, 256]
  S = (Q @ K.T) / sqrt(256)              [R, 4096]
  attn = softmax(S masked by mask)       row-wise
  out = attn @ (diag(w) @ sub)           [R, 256]

Implementation notes:
- Host-side prep is dtype/layout only (no model math): every input is
  pre-packed into the exact [partition, ...] tile-stream layout the kernel
  consumes (bf16, with the 0/1 mask transposed to maskT), so every DMA is a
  big contiguous-per-partition transfer (>=2KB/partition descriptors) and
  all on-device transposes of g/sub and all dtype casts disappear.
- Scores are built TRANSPOSED (s^T [nk, q]) in PSUM so that exp() writes P^T
  directly to SBUF and the second gemm (contraction over nk) needs no
  transposes of P.
- The mask is applied MULTIPLICATIVELY after exp on the DVE:
  pT = exp(s^T) * maskT (0/1). This costs ~5us/macro of DVE (which has
  slack) instead of ~3.4us/macro of PE (the bottleneck). |s| is small
  (~N(0,1)) so the unstabilized exp cannot overflow, and masked-out entries
  are exactly 0.
- The denominator rides along as a 257th "ones" column of V: one extra PSUM
  column per output tile, then a reciprocal multiply on the way out.
- DMA ordering puts the macro-0 critical path first: Wq -> subT (gates the
  K^T build) -> gT0/Wk -> maskT0 -> biases/w -> sub -> maskT1. maskT rides
  the sync HWDGE queue; everything else (incl. output stores) the scalar
  queue. HWDGE issue costs ~630ns per DMA serially, so bulk tensors are
  loaded in as few DMAs as possible.
- Each macro's output gemm is software-pipelined into the next macro's PE
  stream; the last macro's own output gemm is front-loaded into cp>=8 of its
  own score loop (each piece only reads pT columns already written by a
  program-order-earlier exp+mask-mult), leaving only 2 chunks after the
  final exp.
- A PE warm-up burst with no DMA deps issues first thing so the HAM
  clock-gate is less likely to hold the PE at half clock across the
  DMA-dominated ramp-in.
"""
from contextlib import ExitStack

import ml_dtypes
import numpy as np

import concourse.bass as bass
import concourse.tile as tile
from concourse import bacc, masks, mybir
from concourse.bass_utils import run_bass_kernel_spmd

F32 = mybir.dt.float32
BF16 = mybir.dt.bfloat16
AF = mybir.ActivationFunctionType
OP = mybir.AluOpType

NQ, NK = 16384, 4096
QDIM, KDIM, MID = 512, 256, 256
N_CORES = 8
R = NQ // N_CORES            # 2048 rows per core
QMAC = 256                   # q-rows per macro block
NMAC = R // QMAC             # 8
TPM = QMAC // 128            # 2
NKC = NK // 128              # 32

NP_BF16 = ml_dtypes.bfloat16


def _build(loop_n=1):
    nc = bacc.Bacc("TRN2", target_bir_lowering=False, debug=False,
                   num_devices=N_CORES)

    # all inputs host-packed to [128-partition, ...] tile layouts
    gQ_d = nc.dram_tensor("gQ", [NMAC, 128, 4, QMAC], BF16, kind="ExternalInput").ap()
    subQ_d = nc.dram_tensor("subQ", [128, NKC, KDIM], BF16, kind="ExternalInput").ap()
    subTQ_d = nc.dram_tensor("subTQ", [128, 2, NK], BF16, kind="ExternalInput").ap()
    wQ_d = nc.dram_tensor("wQ", [32, 128], F32, kind="ExternalInput").ap()
    maskQ_d = nc.dram_tensor("maskQ", [NMAC, 128, NKC, QMAC], BF16, kind="ExternalInput").ap()
    wqQ_d = nc.dram_tensor("wqQ", [128, 2, QDIM], BF16, kind="ExternalInput").ap()
    bqT_d = nc.dram_tensor("bqT", [128, 2], F32, kind="ExternalInput").ap()
    wkQ_d = nc.dram_tensor("wkQ", [128, 2, KDIM], BF16, kind="ExternalInput").ap()
    bkT_d = nc.dram_tensor("bkT", [128, 2], F32, kind="ExternalInput").ap()
    out_d = nc.dram_tensor("out", [R, KDIM], F32, kind="ExternalOutput").ap()

    with tile.TileContext(nc) as tc, ExitStack() as ctx:
        const = ctx.enter_context(tc.tile_pool(name="const", bufs=1))
        kv = ctx.enter_context(tc.tile_pool(name="kv", bufs=1))
        io = ctx.enter_context(tc.tile_pool(name="io", bufs=3))
        iomt = ctx.enter_context(tc.tile_pool(name="iomt", bufs=3))
        pexp = ctx.enter_context(tc.tile_pool(name="pexp", bufs=3))
        work = ctx.enter_context(tc.tile_pool(name="work", bufs=2))
        prol = ctx.enter_context(tc.tile_pool(name="prol", bufs=1))
        ps_s = ctx.enter_context(tc.tile_pool(name="ps_s", bufs=3, space="PSUM"))
        ps_sm = ctx.enter_context(tc.tile_pool(name="ps_sm", bufs=2, space="PSUM"))
        ps_go = ctx.enter_context(tc.tile_pool(name="ps_go", bufs=3, space="PSUM"))

        loop_cm = tc.For_i(0, loop_n) if loop_n != 1 else None

        def body():
            ident = const.tile([128, 128], BF16, tag="ident")
            masks.make_identity(nc, ident[:])

            def warm_burst(n):
                # HAM warmers: bursts of PE matmuls with no DMA deps. The
                # clock-gate only releases (1.2->2.4 GHz) after ~3.4us of
                # SUSTAINED PE activity, and re-throttles after ~3.4us idle.
                wt = ps_sm.tile([128, 128], F32, tag="ps_small")
                for _ in range(n):
                    nc.tensor.matmul(wt[:], ident[:], ident[:])

            warm_burst(40)

            wqT = const.tile([128, 4, 2, 128], BF16, tag="wqT")
            wkT = const.tile([128, 2, 2, 128], BF16, tag="wkT")
            v_sb = kv.tile([128, NKC, KDIM + 1], BF16, tag="v_sb")
            kT = kv.tile([128, 2, NK], BF16, tag="kT")
            subT_sb = kv.tile([128, 2, NK], BF16, tag="subT_sb")
            sub_sb = kv.tile([128, NKC, KDIM], BF16, tag="sub_sb")

            # ---- prologue DMA issue, critical path first ----
            # scalar HWDGE: tiny weights -> subT -> gT0 (then per-macro gT +
            # out stores). sync HWDGE: maskT0 -> sub -> maskT1 (then per-
            # macro maskT prefetch). Keeping the scalar/ACT queue short means
            # the ACT sequencer (which also runs the kT-build activations and
            # every exp) never head-of-line blocks on a bulk DMA issue.
            wq_bf = prol.tile([128, 2, QDIM], BF16, tag="wq_bf")
            nc.scalar.dma_start(wq_bf[:], wqQ_d)
            wk_bf = prol.tile([128, 2, KDIM], BF16, tag="wk_bf")
            nc.scalar.dma_start(wk_bf[:], wkQ_d)
            bqT = const.tile([128, 2], F32, tag="bqT")
            nc.scalar.dma_start(bqT[:], bqT_d)
            bkT = const.tile([128, 2], F32, tag="bkT")
            nc.scalar.dma_start(bkT[:], bkT_d)
            w_r = prol.tile([32, 128], F32, tag="w_r")
            nc.scalar.dma_start(w_r[:], wQ_d)
            for h in range(2):
                nc.scalar.dma_start(
                    subT_sb[:, :, bass.ts(h, NK // 2)],
                    subTQ_d[:, :, bass.ts(h, NK // 2)],
                )

            g_ts = {}

            def load_gT(mac):
                gTt = io.tile([128, 4, QMAC], BF16, tag="gTt")
                nc.scalar.dma_start(gTt[:], gQ_d[mac])
                g_ts[mac] = gTt

            load_gT(0)

            mask_ts = {}

            def load_maskT(mac):
                mTt = iomt.tile([128, NKC, QMAC], BF16, tag="mTt")
                for h in range(2):
                    nc.sync.dma_start(
                        mTt[:, bass.ts(h, NKC // 2), :],
                        maskQ_d[mac, :, bass.ts(h, NKC // 2), :],
                    )
                mask_ts[mac] = mTt

            load_maskT(0)
            for h in range(2):
                nc.sync.dma_start(
                    sub_sb[:, bass.ts(h, NKC // 2), :],
                    subQ_d[:, bass.ts(h, NKC // 2), :],
                )
            load_maskT(1)

            # ---- weight prep (PE) ----
            for m in range(2):
                pt = ps_sm.tile([128, 4, 128], F32, tag="ps_small")
                for qi in range(4):
                    nc.tensor.matmul(
                        pt[:, qi, :], wq_bf[:, m, bass.ts(qi, 128)], ident[:]
                    )
                nc.vector.tensor_copy(wqT[:, :, m, :], pt[:])
            for m in range(2):
                pt = ps_sm.tile([128, 2, 128], F32, tag="ps_small")
                for kc in range(2):
                    nc.tensor.matmul(
                        pt[:, kc, :], wk_bf[:, m, bass.ts(kc, 128)], ident[:]
                    )
                nc.vector.tensor_copy(wkT[:, :, m, :], pt[:])
            w_rb = prol.tile([32, 128], BF16, tag="w_rb")
            nc.vector.tensor_copy(w_rb[:], w_r[:])
            w_sb = const.tile([128, NKC], F32, tag="w_sb")
            pw = ps_sm.tile([128, NKC], F32, tag="ps_small")
            nc.tensor.matmul(pw[:], w_rb[:], ident[0:32, 0:32])
            nc.vector.tensor_copy(w_sb[:], pw[:])

            nc.gpsimd.memset(v_sb[:, :, KDIM : KDIM + 1], 1.0)

            # ---- K^T build from host-transposed subT (PE + ACT) ----
            for m in range(2):
                for gr in range(NKC // 4):
                    pk = ps_sm.tile([128, 512], F32, tag="ps_small")
                    for kc in range(2):
                        nc.tensor.matmul(
                            pk[:],
                            wkT[:, kc, m, :],
                            subT_sb[:, kc, bass.ts(gr, 512)],
                            start=(kc == 0),
                            stop=(kc == 1),
                        )
                    nc.scalar.activation(
                        kT[:, m, bass.ts(gr, 512)], pk[:], AF.Identity,
                        bias=bkT[:, m : m + 1], scale=1.0,
                    )

            # ---- V build (DVE only) ----
            for c in range(NKC):
                nc.vector.tensor_scalar(
                    v_sb[:, c, 0:KDIM], sub_sb[:, c, :],
                    w_sb[:, c : c + 1], None, OP.mult
                )

            # ------------- main loop over q macros -------------
            pT_prev = None
            for mac in range(NMAC):
                gTt = g_ts.pop(mac)
                if mac + 1 < NMAC:
                    load_gT(mac + 1)

                qT = work.tile([128, 2, QMAC], BF16, tag="qT")
                for m in range(2):
                    pq = ps_sm.tile([128, QMAC], F32, tag="ps_small")
                    for qi in range(4):
                        nc.tensor.matmul(
                            pq[:], wqT[:, qi, m, :], gTt[:, qi, :],
                            start=(qi == 0), stop=(qi == 3),
                        )
                    nc.vector.tensor_scalar(
                        qT[:, m, :], pq[:], bqT[:, m : m + 1], None, OP.add
                    )

                if mac + 2 < NMAC:
                    load_maskT(mac + 2)
                mTt = mask_ts.pop(mac)

                pT = work.tile([128, NKC, QMAC], BF16, tag="pT")

                def g2_piece(pmac, pT_src, t, c, g2_state):
                    # one chunk's worth of an output gemm, interleaved into
                    # this macro's PE stream to avoid an end-of-macro barrier
                    po = g2_state.get(t)
                    if po is None:
                        po = ps_go.tile([128, KDIM + 1], F32, tag="ps_go")
                        g2_state[t] = po
                    nc.tensor.matmul(
                        po[:], pT_src[:, c, bass.ts(t, 128)], v_sb[:, c, :],
                        start=(c == 0), stop=(c == NKC - 1),
                    )
                    if c == NKC - 1:
                        rec = io.tile([128, 1], F32, tag="rec")
                        nc.vector.reciprocal(rec[:], po[:, KDIM : KDIM + 1])
                        o_sb = io.tile([128, KDIM], F32, tag="o_sb")
                        nc.vector.tensor_scalar(
                            o_sb[:], po[:, 0:KDIM], rec[:, 0:1], None, OP.mult
                        )
                        row = pmac * QMAC + t * 128
                        nc.scalar.dma_start(out_d[row : row + 128, :], o_sb[:])

                g2_state = {}
                g2_state_own = {}
                NCP = NKC // 2           # 16 cps, 2 c-chunks each
                per = (TPM * NKC) // NCP  # 4 prev-macro g2 pieces per cp
                # last macro's own g2: front-loaded so only chunks {30,31}
                # remain after the final exp (availability: c <= 2cp+1)
                own_sched = {8: range(0, 6), 15: range(30, 32)}
                for cpq in range(9, 15):
                    own_sched[cpq] = range(6 + (cpq - 9) * 4, 10 + (cpq - 9) * 4)

                for cp in range(NCP):
                    ps = ps_s.tile([128, 2, QMAC], F32, tag="ps_sc")
                    for j in range(2):
                        c = cp * 2 + j
                        for m in range(2):
                            nc.tensor.matmul(
                                ps[:, j, :],
                                kT[:, m, bass.ts(c, 128)],
                                qT[:, m, :],
                                start=(m == 0),
                                stop=(m == 1),
                            )
                    pe_raw = pexp.tile([128, 2, QMAC], BF16, tag="pe_raw")
                    nc.scalar.activation(pe_raw[:], ps[:], AF.Exp, scale=0.0625)
                    nc.vector.tensor_tensor(
                        pT[:, bass.ts(cp, 2), :], pe_raw[:],
                        mTt[:, bass.ts(cp, 2), :], OP.mult,
                    )
                    if mac > 0:
                        for k in range(cp * per, (cp + 1) * per):
                            t, c = divmod(k, NKC)
                            g2_piece(mac - 1, pT_prev, t, c, g2_state)
                    if mac == NMAC - 1 and cp >= NCP // 2:
                        for t in range(TPM):
                            for c in own_sched[cp]:
                                g2_piece(mac, pT, t, c, g2_state_own)

                pT_prev = pT

        if loop_cm is not None:
            with loop_cm:
                body()
        else:
            body()

    nc.compile()
    return nc


def prep_in_maps(inputs):
    """Host-side dtype casts + layout packing + row-sharding (no model math):
    returns the 8 per-core input dicts for _build()'s dram tensors."""
    g = np.asarray(inputs["global_embeddings"]).astype(NP_BF16)      # [NQ, 512]
    sub = np.asarray(inputs["substruct_embeddings"]).astype(NP_BF16)  # [NK, 256]
    w = np.asarray(inputs["substruct_weight"], dtype=np.float32)
    mask = np.asarray(inputs["mask"])
    Wq = np.asarray(inputs["Wq"]).astype(NP_BF16)
    bq = np.asarray(inputs["bq"], dtype=np.float32)
    Wk = np.asarray(inputs["Wk"]).astype(NP_BF16)
    bk = np.asarray(inputs["bk"], dtype=np.float32)

    # shared (replicated) packs
    subQ = np.ascontiguousarray(
        sub.reshape(NKC, 128, KDIM).transpose(1, 0, 2))              # [128, NKC, KDIM]
    subTQ = np.ascontiguousarray(
        sub.T.reshape(2, 128, NK).transpose(1, 0, 2))                # [128, 2, NK]
    wQ = np.ascontiguousarray(w.reshape(32, 128))
    wqQ = np.ascontiguousarray(Wq.reshape(2, 128, QDIM).transpose(1, 0, 2))
    bqT = np.ascontiguousarray(bq.reshape(2, 128).T)
    wkQ = np.ascontiguousarray(Wk.reshape(2, 128, KDIM).transpose(1, 0, 2))
    bkT = np.ascontiguousarray(bk.reshape(2, 128).T)

    # maskQ[mac, p, c, q] = mask[core*R + mac*QMAC + q, c*128 + p]  (bf16 0/1)
    maskT = mask.T.astype(NP_BF16)                                    # [NK, NQ]
    gT = g.T                                                          # [512, NQ]

    in_maps = []
    for i in range(N_CORES):
        sl = slice(i * R, (i + 1) * R)
        mTc = maskT[:, sl]                                            # [NK, R]
        maskQ = np.ascontiguousarray(
            mTc.reshape(NKC, 128, NMAC, QMAC).transpose(2, 1, 0, 3))  # [NMAC,128,NKC,QMAC]
        gTc = gT[:, sl]                                               # [512, R]
        gQ = np.ascontiguousarray(
            gTc.reshape(4, 128, NMAC, QMAC).transpose(2, 1, 0, 3))    # [NMAC,128,4,QMAC]
        in_maps.append({
            "gQ": gQ, "maskQ": maskQ,
            "subQ": subQ, "subTQ": subTQ, "wQ": wQ,
            "wqQ": wqQ, "bqT": bqT, "wkQ": wkQ, "bkT": bkT,
        })
    return in_maps


_CACHE = {}


def kernel(**inputs) -> np.ndarray:
    """Full-input entry point: shards NQ across 8 NeuronCores, runs the Bass
    kernel, and gathers the full [16384, 256] float32 output."""
    if "nc" not in _CACHE:
        _CACHE["nc"] = _build()
    nc = _CACHE["nc"]

    in_maps = prep_in_maps(inputs)
    res = run_bass_kernel_spmd(nc, in_maps, list(range(N_CORES))).results
    return np.concatenate([res[i]["out"] for i in range(N_CORES)], axis=0)


# revision 22
# speedup vs baseline: 1.3223x; 1.0284x over previous
"""Trainium2 Bass kernel for nn_AdjAttenAgger (masked cross-attention
aggregation), running SPMD on 8 NeuronCores.

Math (row-sharded 8 ways over NQ=16384):
  Q = g @ Wq.T + bq                      [R, 256]
  K = sub @ Wk.T + bk                    [4096, 256]
  S = (Q @ K.T) / sqrt(256)              [R, 4096]
  attn = softmax(S masked by mask)       row-wise
  out = attn @ (diag(w) @ sub)           [R, 256]

Implementation notes:
- Host-side prep is dtype/layout only (no model math): every input is
  pre-packed into the exact [partition, ...] tile-stream layout the kernel
  consumes (bf16, with the 0/1 mask transposed to maskT), so every DMA is a
  big contiguous-per-partition transfer (>=2KB/partition descriptors) and
  all on-device transposes of g/sub and all dtype casts disappear.
- Scores are built TRANSPOSED (s^T [nk, q]) in PSUM so that exp() writes P^T
  directly to SBUF and the second gemm (contraction over nk) needs no
  transposes of P.
- The mask is applied MULTIPLICATIVELY after exp on the DVE:
  pT = exp(s^T) * maskT (0/1). This costs ~5us/macro of DVE (which has
  slack) instead of ~3.4us/macro of PE (the bottleneck). |s| is small
  (~N(0,1)) so the unstabilized exp cannot overflow, and masked-out entries
  are exactly 0.
- The denominator rides along as a 257th "ones" column of V: one extra PSUM
  column per output tile, then a reciprocal multiply on the way out.
- DMA ordering puts the macro-0 critical path first: Wq -> subT (gates the
  K^T build) -> gT0/Wk -> maskT0 -> biases/w -> sub -> maskT1. maskT rides
  the sync HWDGE queue; everything else (incl. output stores) the scalar
  queue. HWDGE issue costs ~630ns per DMA serially, so bulk tensors are
  loaded in as few DMAs as possible.
- Each macro's output gemm is software-pipelined into the next macro's PE
  stream; the last macro's own output gemm is front-loaded into cp>=8 of its
  own score loop (each piece only reads pT columns already written by a
  program-order-earlier exp+mask-mult), leaving only 2 chunks after the
  final exp.
- A PE warm-up burst with no DMA deps issues first thing so the HAM
  clock-gate is less likely to hold the PE at half clock across the
  DMA-dominated ramp-in.
"""
from contextlib import ExitStack

import ml_dtypes
import numpy as np

import concourse.bass as bass
import concourse.tile as tile
from concourse import bacc, masks, mybir
from concourse.bass_utils import run_bass_kernel_spmd

F32 = mybir.dt.float32
BF16 = mybir.dt.bfloat16
AF = mybir.ActivationFunctionType
OP = mybir.AluOpType

NQ, NK = 16384, 4096
QDIM, KDIM, MID = 512, 256, 256
N_CORES = 8
R = NQ // N_CORES            # 2048 rows per core
QMAC = 256                   # q-rows per macro block
NMAC = R // QMAC             # 8
TPM = QMAC // 128            # 2
NKC = NK // 128              # 32

NP_BF16 = ml_dtypes.bfloat16


def _build(loop_n=1):
    nc = bacc.Bacc("TRN2", target_bir_lowering=False, debug=False,
                   num_devices=N_CORES)

    # all inputs host-packed to [128-partition, ...] tile layouts
    gQ_d = nc.dram_tensor("gQ", [NMAC, 128, 4, QMAC], BF16, kind="ExternalInput").ap()
    subQ_d = nc.dram_tensor("subQ", [128, NKC, KDIM], BF16, kind="ExternalInput").ap()
    subTQ_d = nc.dram_tensor("subTQ", [128, 2, NK], BF16, kind="ExternalInput").ap()
    wQ_d = nc.dram_tensor("wQ", [32, 128], F32, kind="ExternalInput").ap()
    maskQ_d = nc.dram_tensor("maskQ", [NMAC, 128, NKC, QMAC], BF16, kind="ExternalInput").ap()
    wqQ_d = nc.dram_tensor("wqQ", [128, 2, QDIM], BF16, kind="ExternalInput").ap()
    bqT_d = nc.dram_tensor("bqT", [128, 2], F32, kind="ExternalInput").ap()
    wkQ_d = nc.dram_tensor("wkQ", [128, 2, KDIM], BF16, kind="ExternalInput").ap()
    bkT_d = nc.dram_tensor("bkT", [128, 2], F32, kind="ExternalInput").ap()
    out_d = nc.dram_tensor("out", [R, KDIM], BF16, kind="ExternalOutput").ap()

    with tile.TileContext(nc) as tc, ExitStack() as ctx:
        const = ctx.enter_context(tc.tile_pool(name="const", bufs=1))
        kv = ctx.enter_context(tc.tile_pool(name="kv", bufs=1))
        io = ctx.enter_context(tc.tile_pool(name="io", bufs=3))
        iomt = ctx.enter_context(tc.tile_pool(name="iomt", bufs=3))
        pexp = ctx.enter_context(tc.tile_pool(name="pexp", bufs=3))
        work = ctx.enter_context(tc.tile_pool(name="work", bufs=2))
        prol = ctx.enter_context(tc.tile_pool(name="prol", bufs=1))
        ps_s = ctx.enter_context(tc.tile_pool(name="ps_s", bufs=3, space="PSUM"))
        ps_sm = ctx.enter_context(tc.tile_pool(name="ps_sm", bufs=2, space="PSUM"))
        ps_go = ctx.enter_context(tc.tile_pool(name="ps_go", bufs=3, space="PSUM"))

        loop_cm = tc.For_i(0, loop_n) if loop_n != 1 else None

        def body():
            ident = const.tile([128, 128], BF16, tag="ident")
            masks.make_identity(nc, ident[:])

            def warm_burst(n):
                # HAM warmers: bursts of PE matmuls with no DMA deps. The
                # clock-gate only releases (1.2->2.4 GHz) after ~3.4us of
                # SUSTAINED PE activity, and re-throttles after ~3.4us idle.
                wt = ps_sm.tile([128, 128], F32, tag="ps_small")
                for _ in range(n):
                    nc.tensor.matmul(wt[:], ident[:], ident[:])

            warm_burst(32)

            wqT = const.tile([128, 4, 2, 128], BF16, tag="wqT")
            wkT = const.tile([128, 2, 2, 128], BF16, tag="wkT")
            v_sb = kv.tile([128, NKC, KDIM + 1], BF16, tag="v_sb")
            kT = kv.tile([128, 2, NK], BF16, tag="kT")
            subT_sb = kv.tile([128, 2, NK], BF16, tag="subT_sb")
            sub_sb = kv.tile([128, NKC, KDIM], BF16, tag="sub_sb")

            # ---- prologue DMA issue, critical path first ----
            # scalar HWDGE: tiny weights -> subT -> gT0 (then per-macro gT +
            # out stores). sync HWDGE: maskT0 -> sub -> maskT1 (then per-
            # macro maskT prefetch). Keeping the scalar/ACT queue short means
            # the ACT sequencer (which also runs the kT-build activations and
            # every exp) never head-of-line blocks on a bulk DMA issue.
            wq_bf = prol.tile([128, 2, QDIM], BF16, tag="wq_bf")
            nc.scalar.dma_start(wq_bf[:], wqQ_d)
            wk_bf = prol.tile([128, 2, KDIM], BF16, tag="wk_bf")
            nc.scalar.dma_start(wk_bf[:], wkQ_d)
            bqT = const.tile([128, 2], F32, tag="bqT")
            nc.scalar.dma_start(bqT[:], bqT_d)
            bkT = const.tile([128, 2], F32, tag="bkT")
            nc.scalar.dma_start(bkT[:], bkT_d)
            w_r = prol.tile([32, 128], F32, tag="w_r")
            nc.scalar.dma_start(w_r[:], wQ_d)
            # subT halves split across BOTH HWDGE queues so the kT build's
            # gating input doesn't sit behind bulk maskT traffic
            nc.scalar.dma_start(
                subT_sb[:, :, 0 : NK // 2], subTQ_d[:, :, 0 : NK // 2])
            nc.sync.dma_start(
                subT_sb[:, :, NK // 2 : NK], subTQ_d[:, :, NK // 2 : NK])

            g_ts = {}

            def load_gT(mac):
                gTt = io.tile([128, 4, QMAC], BF16, tag="gTt")
                nc.scalar.dma_start(gTt[:], gQ_d[mac])
                g_ts[mac] = gTt

            load_gT(0)

            mask_ts = {}

            def load_maskT(mac):
                mTt = iomt.tile([128, NKC, QMAC], BF16, tag="mTt")
                for h in range(2):
                    nc.sync.dma_start(
                        mTt[:, bass.ts(h, NKC // 2), :],
                        maskQ_d[mac, :, bass.ts(h, NKC // 2), :],
                    )
                mask_ts[mac] = mTt

            load_maskT(0)
            for h in range(2):
                nc.sync.dma_start(
                    sub_sb[:, bass.ts(h, NKC // 2), :],
                    subQ_d[:, bass.ts(h, NKC // 2), :],
                )
            load_maskT(1)

            # ---- weight prep (PE) ----
            for m in range(2):
                pt = ps_sm.tile([128, 4, 128], F32, tag="ps_small")
                for qi in range(4):
                    nc.tensor.matmul(
                        pt[:, qi, :], wq_bf[:, m, bass.ts(qi, 128)], ident[:]
                    )
                nc.vector.tensor_copy(wqT[:, :, m, :], pt[:])
            for m in range(2):
                pt = ps_sm.tile([128, 2, 128], F32, tag="ps_small")
                for kc in range(2):
                    nc.tensor.matmul(
                        pt[:, kc, :], wk_bf[:, m, bass.ts(kc, 128)], ident[:]
                    )
                nc.vector.tensor_copy(wkT[:, :, m, :], pt[:])
            w_rb = prol.tile([32, 128], BF16, tag="w_rb")
            nc.vector.tensor_copy(w_rb[:], w_r[:])
            w_sb = const.tile([128, NKC], F32, tag="w_sb")
            pw = ps_sm.tile([128, NKC], F32, tag="ps_small")
            nc.tensor.matmul(pw[:], w_rb[:], ident[0:32, 0:32])
            nc.vector.tensor_copy(w_sb[:], pw[:])

            nc.gpsimd.memset(v_sb[:, :, KDIM : KDIM + 1], 1.0)

            # ---- K^T build from host-transposed subT (PE + ACT) ----
            for gr_m in [(gr, m) for gr in (0, 1, 2, 3) for m in (0, 1)] + \
                        [(gr, m) for gr in (4, 5, 6, 7) for m in (0, 1)]:
                    gr, m = gr_m
                    pool, ptag = ((ps_sm, "ps_small") if gr % 2 == 0
                                  else (ps_go, "ps_go"))
                    pk = pool.tile([128, 512], F32, tag=ptag)
                    for kc in range(2):
                        nc.tensor.matmul(
                            pk[:],
                            wkT[:, kc, m, :],
                            subT_sb[:, kc, bass.ts(gr, 512)],
                            start=(kc == 0),
                            stop=(kc == 1),
                        )
                    if gr % 2 == 0:
                        nc.scalar.activation(
                            kT[:, m, bass.ts(gr, 512)], pk[:], AF.Identity,
                            bias=bkT[:, m : m + 1], scale=1.0,
                        )
                    else:
                        nc.vector.tensor_scalar(
                            kT[:, m, bass.ts(gr, 512)], pk[:],
                            bkT[:, m : m + 1], None, OP.add
                        )

            # ---- V build (split ACT/DVE; both have prologue slack) ----
            for c in range(NKC):
                if c % 2 == 0:
                    nc.scalar.activation(
                        v_sb[:, c, 0:KDIM], sub_sb[:, c, :], AF.Copy,
                        scale=w_sb[:, c : c + 1],
                    )
                else:
                    nc.vector.tensor_scalar(
                        v_sb[:, c, 0:KDIM], sub_sb[:, c, :],
                        w_sb[:, c : c + 1], None, OP.mult
                    )

            # ------------- main loop over q macros -------------
            pT_prev = None
            for mac in range(NMAC):
                gTt = g_ts.pop(mac)
                if mac + 1 < NMAC:
                    load_gT(mac + 1)

                qT = work.tile([128, 2, QMAC], BF16, tag="qT")
                for m in range(2):
                    pq = ps_sm.tile([128, QMAC], F32, tag="ps_small")
                    for qi in range(4):
                        nc.tensor.matmul(
                            pq[:], wqT[:, qi, m, :], gTt[:, qi, :],
                            start=(qi == 0), stop=(qi == 3),
                        )
                    nc.vector.tensor_scalar(
                        qT[:, m, :], pq[:], bqT[:, m : m + 1], None, OP.add
                    )

                if mac + 2 < NMAC:
                    load_maskT(mac + 2)
                mTt = mask_ts.pop(mac)

                pT = work.tile([128, NKC, QMAC], BF16, tag="pT")

                def g2_piece(pmac, pT_src, t, c, g2_state):
                    # one chunk's worth of an output gemm, interleaved into
                    # this macro's PE stream to avoid an end-of-macro barrier
                    po = g2_state.get(t)
                    if po is None:
                        po = ps_go.tile([128, KDIM + 1], F32, tag="ps_go")
                        g2_state[t] = po
                    nc.tensor.matmul(
                        po[:], pT_src[:, c, bass.ts(t, 128)], v_sb[:, c, :],
                        start=(c == 0), stop=(c == NKC - 1),
                    )
                    if c == NKC - 1:
                        rec = io.tile([128, 1], F32, tag="rec")
                        nc.vector.reciprocal(rec[:], po[:, KDIM : KDIM + 1])
                        o_sb = io.tile([128, KDIM], BF16, tag="o_sb")
                        nc.vector.tensor_scalar(
                            o_sb[:], po[:, 0:KDIM], rec[:, 0:1], None, OP.mult
                        )
                        row = pmac * QMAC + t * 128
                        nc.scalar.dma_start(out_d[row : row + 128, :], o_sb[:])

                g2_state = {}
                g2_state_own = {}
                NCP = NKC // 2           # 16 cps, 2 c-chunks each
                per = (TPM * NKC) // NCP  # 4 prev-macro g2 pieces per cp
                # last macro's own g2: front-loaded so only chunks {30,31}
                # remain after the final exp (availability: c <= 2cp+1)
                own_sched = {8: range(0, 6), 15: range(30, 32)}
                for cpq in range(9, 15):
                    own_sched[cpq] = range(6 + (cpq - 9) * 4, 10 + (cpq - 9) * 4)

                for cp in range(NCP):
                    ps = ps_s.tile([128, 2, QMAC], F32, tag="ps_sc")
                    for j in range(2):
                        c = cp * 2 + j
                        for m in range(2):
                            nc.tensor.matmul(
                                ps[:, j, :],
                                kT[:, m, bass.ts(c, 128)],
                                qT[:, m, :],
                                start=(m == 0),
                                stop=(m == 1),
                            )
                    pe_raw = pexp.tile([128, 2, QMAC], BF16, tag="pe_raw")
                    nc.scalar.activation(pe_raw[:], ps[:], AF.Exp, scale=0.0625)
                    nc.vector.tensor_tensor(
                        pT[:, bass.ts(cp, 2), :], pe_raw[:],
                        mTt[:, bass.ts(cp, 2), :], OP.mult,
                    )
                    if mac > 0:
                        for k in range(cp * per, (cp + 1) * per):
                            t, c = divmod(k, NKC)
                            g2_piece(mac - 1, pT_prev, t, c, g2_state)
                    if mac == NMAC - 1 and cp >= NCP // 2:
                        for t in range(TPM):
                            for c in own_sched[cp]:
                                g2_piece(mac, pT, t, c, g2_state_own)

                pT_prev = pT

        if loop_cm is not None:
            with loop_cm:
                body()
        else:
            body()

    nc.compile()
    return nc


def prep_in_maps(inputs):
    """Host-side dtype casts + layout packing + row-sharding (no model math):
    returns the 8 per-core input dicts for _build()'s dram tensors."""
    g = np.asarray(inputs["global_embeddings"]).astype(NP_BF16)      # [NQ, 512]
    sub = np.asarray(inputs["substruct_embeddings"]).astype(NP_BF16)  # [NK, 256]
    w = np.asarray(inputs["substruct_weight"], dtype=np.float32)
    mask = np.asarray(inputs["mask"])
    Wq = np.asarray(inputs["Wq"]).astype(NP_BF16)
    bq = np.asarray(inputs["bq"], dtype=np.float32)
    Wk = np.asarray(inputs["Wk"]).astype(NP_BF16)
    bk = np.asarray(inputs["bk"], dtype=np.float32)

    # shared (replicated) packs
    subQ = np.ascontiguousarray(
        sub.reshape(NKC, 128, KDIM).transpose(1, 0, 2))              # [128, NKC, KDIM]
    subTQ = np.ascontiguousarray(
        sub.T.reshape(2, 128, NK).transpose(1, 0, 2))                # [128, 2, NK]
    wQ = np.ascontiguousarray(w.reshape(32, 128))
    wqQ = np.ascontiguousarray(Wq.reshape(2, 128, QDIM).transpose(1, 0, 2))
    bqT = np.ascontiguousarray(bq.reshape(2, 128).T)
    wkQ = np.ascontiguousarray(Wk.reshape(2, 128, KDIM).transpose(1, 0, 2))
    bkT = np.ascontiguousarray(bk.reshape(2, 128).T)

    # maskQ[mac, p, c, q] = mask[core*R + mac*QMAC + q, c*128 + p]  (bf16 0/1)
    maskT = mask.T.astype(NP_BF16)                                    # [NK, NQ]
    gT = g.T                                                          # [512, NQ]

    in_maps = []
    for i in range(N_CORES):
        sl = slice(i * R, (i + 1) * R)
        mTc = maskT[:, sl]                                            # [NK, R]
        maskQ = np.ascontiguousarray(
            mTc.reshape(NKC, 128, NMAC, QMAC).transpose(2, 1, 0, 3))  # [NMAC,128,NKC,QMAC]
        gTc = gT[:, sl]                                               # [512, R]
        gQ = np.ascontiguousarray(
            gTc.reshape(4, 128, NMAC, QMAC).transpose(2, 1, 0, 3))    # [NMAC,128,4,QMAC]
        in_maps.append({
            "gQ": gQ, "maskQ": maskQ,
            "subQ": subQ, "subTQ": subTQ, "wQ": wQ,
            "wqQ": wqQ, "bqT": bqT, "wkQ": wkQ, "bkT": bkT,
        })
    return in_maps


_CACHE = {}


def kernel(**inputs) -> np.ndarray:
    """Full-input entry point: shards NQ across 8 NeuronCores, runs the Bass
    kernel, and gathers the full [16384, 256] float32 output."""
    if "nc" not in _CACHE:
        _CACHE["nc"] = _build()
    nc = _CACHE["nc"]

    in_maps = prep_in_maps(inputs)
    res = run_bass_kernel_spmd(nc, in_maps, list(range(N_CORES))).results
    return np.concatenate(
        [res[i]["out"] for i in range(N_CORES)], axis=0
    ).astype(np.float32)


# revision 29
# speedup vs baseline: 1.3248x; 1.0019x over previous
"""Trainium2 Bass kernel for nn_AdjAttenAgger (masked cross-attention
aggregation), running SPMD on 8 NeuronCores.

Math (row-sharded 8 ways over NQ=16384):
  Q = g @ Wq.T + bq                      [R, 256]
  K = sub @ Wk.T + bk                    [4096, 256]
  S = (Q @ K.T) / sqrt(256)              [R, 4096]
  attn = softmax(S masked by mask)       row-wise
  out = attn @ (diag(w) @ sub)           [R, 256]

Implementation notes:
- Host-side prep is dtype/layout only (no model math): every input is
  pre-packed into the exact [partition, ...] tile-stream layout the kernel
  consumes (bf16, with the 0/1 mask transposed to maskT), so every DMA is a
  big contiguous-per-partition transfer (>=2KB/partition descriptors) and
  all on-device transposes of g/sub and all dtype casts disappear.
- Scores are built TRANSPOSED (s^T [nk, q]) in PSUM so that exp() writes P^T
  directly to SBUF and the second gemm (contraction over nk) needs no
  transposes of P.
- The mask is applied MULTIPLICATIVELY after exp on the DVE:
  pT = exp(s^T) * maskT (0/1). This costs ~5us/macro of DVE (which has
  slack) instead of ~3.4us/macro of PE (the bottleneck). |s| is small
  (~N(0,1)) so the unstabilized exp cannot overflow, and masked-out entries
  are exactly 0.
- The denominator rides along as a 257th "ones" column of V: one extra PSUM
  column per output tile, then a reciprocal multiply on the way out.
- DMA ordering puts the macro-0 critical path first: Wq -> subT (gates the
  K^T build) -> gT0/Wk -> maskT0 -> biases/w -> sub -> maskT1. maskT rides
  the sync HWDGE queue; everything else (incl. output stores) the scalar
  queue. HWDGE issue costs ~630ns per DMA serially, so bulk tensors are
  loaded in as few DMAs as possible.
- Each macro's output gemm is software-pipelined into the next macro's PE
  stream; the last macro's own output gemm is front-loaded into cp>=8 of its
  own score loop (each piece only reads pT columns already written by a
  program-order-earlier exp+mask-mult), leaving only 2 chunks after the
  final exp.
- A PE warm-up burst with no DMA deps issues first thing so the HAM
  clock-gate is less likely to hold the PE at half clock across the
  DMA-dominated ramp-in.
"""
from contextlib import ExitStack

import ml_dtypes
import numpy as np

import concourse.bass as bass
import concourse.tile as tile
from concourse import bacc, masks, mybir
from concourse.bass_utils import run_bass_kernel_spmd

F32 = mybir.dt.float32
BF16 = mybir.dt.bfloat16
FP8 = mybir.dt.float8e4
PM = mybir.MatmulPerfMode
AF = mybir.ActivationFunctionType
OP = mybir.AluOpType

NQ, NK = 16384, 4096
QDIM, KDIM, MID = 512, 256, 256
N_CORES = 8
R = NQ // N_CORES            # 2048 rows per core
QMAC = 256                   # q-rows per macro block
NMAC = R // QMAC             # 8
TPM = QMAC // 128            # 2
NKC = NK // 128              # 32

NP_BF16 = ml_dtypes.bfloat16


def _build(loop_n=1):
    nc = bacc.Bacc("TRN2", target_bir_lowering=False, debug=False,
                   num_devices=N_CORES)

    # all inputs host-packed to [128-partition, ...] tile layouts
    gQ_d = nc.dram_tensor("gQ", [NMAC, 128, 4, QMAC], BF16, kind="ExternalInput").ap()
    subQ_d = nc.dram_tensor("subQ", [128, NKC, KDIM], BF16, kind="ExternalInput").ap()
    subTQ_d = nc.dram_tensor("subTQ", [128, 2, NK], BF16, kind="ExternalInput").ap()
    wQ_d = nc.dram_tensor("wQ", [32, 128], F32, kind="ExternalInput").ap()
    maskQ_d = nc.dram_tensor("maskQ", [NMAC, 128, NKC, QMAC], BF16, kind="ExternalInput").ap()
    wqQ_d = nc.dram_tensor("wqQ", [128, 2, QDIM], BF16, kind="ExternalInput").ap()
    bqT_d = nc.dram_tensor("bqT", [128, 2], F32, kind="ExternalInput").ap()
    wkQ_d = nc.dram_tensor("wkQ", [128, 2, KDIM], BF16, kind="ExternalInput").ap()
    bkT_d = nc.dram_tensor("bkT", [128, 2], F32, kind="ExternalInput").ap()
    out_d = nc.dram_tensor("out", [R, KDIM], BF16, kind="ExternalOutput").ap()

    with tile.TileContext(nc) as tc, ExitStack() as ctx:
        const = ctx.enter_context(tc.tile_pool(name="const", bufs=1))
        kv = ctx.enter_context(tc.tile_pool(name="kv", bufs=1))
        io = ctx.enter_context(tc.tile_pool(name="io", bufs=3))
        iomt = ctx.enter_context(tc.tile_pool(name="iomt", bufs=3))
        pexp = ctx.enter_context(tc.tile_pool(name="pexp", bufs=3))
        work = ctx.enter_context(tc.tile_pool(name="work", bufs=2))
        prol = ctx.enter_context(tc.tile_pool(name="prol", bufs=1))
        ps_s = ctx.enter_context(tc.tile_pool(name="ps_s", bufs=2, space="PSUM"))
        ps_sm = ctx.enter_context(tc.tile_pool(name="ps_sm", bufs=2, space="PSUM"))
        ps_go = ctx.enter_context(tc.tile_pool(name="ps_go", bufs=2, space="PSUM"))

        loop_cm = tc.For_i(0, loop_n) if loop_n != 1 else None

        def body():
            ident = const.tile([128, 128], BF16, tag="ident")
            masks.make_identity(nc, ident[:])

            def warm_burst(n):
                # HAM warmers: bursts of PE matmuls with no DMA deps. The
                # clock-gate only releases (1.2->2.4 GHz) after ~3.4us of
                # SUSTAINED PE activity, and re-throttles after ~3.4us idle.
                wt = ps_sm.tile([128, 128], F32, tag="ps_small")
                for _ in range(n):
                    nc.tensor.matmul(wt[:], ident[:], ident[:])

            warm_burst(16)

            wqT = const.tile([128, 4, 2, 128], BF16, tag="wqT")
            wkT = const.tile([128, 2, 2, 128], BF16, tag="wkT")
            v_sb = kv.tile([128, NKC, KDIM + 1], BF16, tag="v_sb")
            kT = kv.tile([128, 2, NK], BF16, tag="kT")
            subT_sb = kv.tile([128, 2, NK], BF16, tag="subT_sb")
            sub_sb = kv.tile([128, NKC, KDIM], BF16, tag="sub_sb")

            # ---- prologue DMA issue, critical path first ----
            # scalar HWDGE: tiny weights -> subT -> gT0 (then per-macro gT +
            # out stores). sync HWDGE: maskT0 -> sub -> maskT1 (then per-
            # macro maskT prefetch). Keeping the scalar/ACT queue short means
            # the ACT sequencer (which also runs the kT-build activations and
            # every exp) never head-of-line blocks on a bulk DMA issue.
            wq_bf = prol.tile([128, 2, QDIM], BF16, tag="wq_bf")
            nc.scalar.dma_start(wq_bf[:], wqQ_d)
            wk_bf = prol.tile([128, 2, KDIM], BF16, tag="wk_bf")
            nc.scalar.dma_start(wk_bf[:], wkQ_d)
            bqT = const.tile([128, 2], F32, tag="bqT")
            nc.scalar.dma_start(bqT[:], bqT_d)
            bkT = const.tile([128, 2], F32, tag="bkT")
            nc.scalar.dma_start(bkT[:], bkT_d)
            w_r = prol.tile([32, 128], F32, tag="w_r")
            nc.scalar.dma_start(w_r[:], wQ_d)
            # subT halves split across BOTH HWDGE queues so the kT build's
            # gating input doesn't sit behind bulk maskT traffic
            nc.scalar.dma_start(
                subT_sb[:, :, 0 : NK // 2], subTQ_d[:, :, 0 : NK // 2])
            nc.sync.dma_start(
                subT_sb[:, :, NK // 2 : NK], subTQ_d[:, :, NK // 2 : NK])

            g_ts = {}

            def load_gT(mac):
                gTt = io.tile([128, 4, QMAC], BF16, tag="gTt")
                nc.scalar.dma_start(gTt[:], gQ_d[mac])
                g_ts[mac] = gTt

            load_gT(0)

            mask_ts = {}

            def load_maskT(mac):
                mTt = iomt.tile([128, NKC, QMAC], BF16, tag="mTt")
                for h in range(2):
                    nc.sync.dma_start(
                        mTt[:, bass.ts(h, NKC // 2), :],
                        maskQ_d[mac, :, bass.ts(h, NKC // 2), :],
                    )
                mask_ts[mac] = mTt

            load_maskT(0)
            for h in range(2):
                nc.sync.dma_start(
                    sub_sb[:, bass.ts(h, NKC // 2), :],
                    subQ_d[:, bass.ts(h, NKC // 2), :],
                )
            load_maskT(1)

            # ---- weight prep (PE) ----
            for m in range(2):
                pt = ps_sm.tile([128, 4, 128], F32, tag="ps_small")
                for qi in range(4):
                    nc.tensor.matmul(
                        pt[:, qi, :], wq_bf[:, m, bass.ts(qi, 128)], ident[:]
                    )
                nc.vector.tensor_copy(wqT[:, :, m, :], pt[:])
            for m in range(2):
                pt = ps_sm.tile([128, 2, 128], F32, tag="ps_small")
                for kc in range(2):
                    nc.tensor.matmul(
                        pt[:, kc, :], wk_bf[:, m, bass.ts(kc, 128)], ident[:]
                    )
                nc.vector.tensor_copy(wkT[:, :, m, :], pt[:])
            w_rb = prol.tile([32, 128], BF16, tag="w_rb")
            nc.vector.tensor_copy(w_rb[:], w_r[:])
            w_sb = const.tile([128, NKC], F32, tag="w_sb")
            pw = ps_sm.tile([128, NKC], F32, tag="ps_small")
            nc.tensor.matmul(pw[:], w_rb[:], ident[0:32, 0:32])
            nc.vector.tensor_copy(w_sb[:], pw[:])

            nc.gpsimd.memset(v_sb[:, :, KDIM : KDIM + 1], 1.0)
            warm_burst(24)

            # ---- K^T build from host-transposed subT (PE + ACT) ----
            for gr_m in [(gr, m) for gr in (0, 1, 2, 3) for m in (0, 1)] + \
                        [(gr, m) for gr in (4, 5, 6, 7) for m in (0, 1)]:
                    gr, m = gr_m
                    pool, ptag = ((ps_sm, "ps_small") if gr % 2 == 0
                                  else (ps_go, "ps_go"))
                    pk = pool.tile([128, 512], F32, tag=ptag)
                    for kc in range(2):
                        nc.tensor.matmul(
                            pk[:],
                            wkT[:, kc, m, :],
                            subT_sb[:, kc, bass.ts(gr, 512)],
                            start=(kc == 0),
                            stop=(kc == 1),
                        )
                    if gr % 2 == 0:
                        nc.scalar.activation(
                            kT[:, m, bass.ts(gr, 512)], pk[:], AF.Identity,
                            bias=bkT[:, m : m + 1], scale=1.0,
                        )
                    else:
                        nc.vector.tensor_scalar(
                            kT[:, m, bass.ts(gr, 512)], pk[:],
                            bkT[:, m : m + 1], None, OP.add
                        )


            # ------------- main loop over q macros -------------
            pT_prev = None
            for mac in range(NMAC):
                gTt = g_ts.pop(mac)
                if mac + 1 < NMAC:
                    load_gT(mac + 1)

                qT = work.tile([128, 2, QMAC], BF16, tag="qT")
                for m in range(2):
                    pq = ps_sm.tile([128, QMAC], F32, tag="ps_small")
                    for qi in range(4):
                        nc.tensor.matmul(
                            pq[:], wqT[:, qi, m, :], gTt[:, qi, :],
                            start=(qi == 0), stop=(qi == 3),
                        )
                    nc.vector.tensor_scalar(
                        qT[:, m, :], pq[:], bqT[:, m : m + 1], None, OP.add
                    )

                if mac + 2 < NMAC:
                    load_maskT(mac + 2)
                mTt = mask_ts.pop(mac)

                pT = work.tile([128, NKC, QMAC], BF16, tag="pT")

                def g2_piece(pmac, pT_src, t, c, g2_state, own=False):
                    # one chunk's worth of an output gemm, interleaved into
                    # this macro's PE stream to avoid an end-of-macro barrier
                    po = g2_state.get(t)
                    if po is None:
                        if own:
                            po = ps_sm.tile([128, KDIM + 1], F32, tag="ps_small")
                        else:
                            po = ps_go.tile([128, KDIM + 1], F32, tag="ps_go")
                        g2_state[t] = po
                    nc.tensor.matmul(
                        po[:], pT_src[:, c, bass.ts(t, 128)], v_sb[:, c, :],
                        start=(c == 0), stop=(c == NKC - 1),
                    )
                    if c == NKC - 1:
                        rec = io.tile([128, 1], F32, tag="rec")
                        nc.vector.reciprocal(rec[:], po[:, KDIM : KDIM + 1])
                        o_sb = io.tile([128, KDIM], BF16, tag="o_sb")
                        nc.vector.tensor_scalar(
                            o_sb[:], po[:, 0:KDIM], rec[:, 0:1], None, OP.mult
                        )
                        row = pmac * QMAC + t * 128
                        nc.scalar.dma_start(out_d[row : row + 128, :], o_sb[:])

                g2_state = {}
                g2_state_own = {}
                NSS = NKC // 4           # 8 supersteps, 4 c-chunks each
                per = (TPM * NKC) // NSS  # 8 prev-macro g2 pieces per ss
                # last macro's own g2: front-loaded under the availability
                # constraint c <= 4ss+3, tail = chunks {28..31}
                own_sched = {4: range(0, 12), 5: range(12, 20),
                             6: range(20, 28), 7: range(31, 32)}

                last_mac = mac == NMAC - 1
                for ss in range(NSS):
                    ps = ps_s.tile([128, 4, QMAC], F32, tag="ps_sc")
                    for j in range(4):
                        c = ss * 4 + j
                        for m in range(2):
                            nc.tensor.matmul(
                                ps[:, j, :],
                                kT[:, m, bass.ts(c, 128)],
                                qT[:, m, :],
                                start=(m == 0),
                                stop=(m == 1),
                            )
                    pe_raw = pexp.tile([128, 4, QMAC], BF16, tag="pe_raw")
                    if last_mac and ss == NSS - 1:
                        # split the final exp/mult so only chunk 31's tiny
                        # tail remains after the last activation
                        nc.scalar.activation(
                            pe_raw[:, 0:3, :], ps[:, 0:3, :], AF.Exp,
                            scale=0.0625)
                        nc.vector.tensor_tensor(
                            pT[:, ss * 4 : ss * 4 + 3, :], pe_raw[:, 0:3, :],
                            mTt[:, ss * 4 : ss * 4 + 3, :], OP.mult,
                        )
                        for t in range(TPM):
                            for c in range(28, 31):
                                g2_piece(mac, pT, t, c, g2_state_own, own=True)
                        nc.scalar.activation(
                            pe_raw[:, 3:4, :], ps[:, 3:4, :], AF.Exp,
                            scale=0.0625)
                        nc.vector.tensor_tensor(
                            pT[:, ss * 4 + 3 : ss * 4 + 4, :],
                            pe_raw[:, 3:4, :],
                            mTt[:, ss * 4 + 3 : ss * 4 + 4, :], OP.mult,
                        )
                    else:
                        nc.scalar.activation(
                            pe_raw[:], ps[:], AF.Exp, scale=0.0625)
                        nc.vector.tensor_tensor(
                            pT[:, bass.ts(ss, 4), :], pe_raw[:],
                            mTt[:, bass.ts(ss, 4), :], OP.mult,
                        )
                    if mac == 0:
                        # V build rides macro 0's DVE stream, one ss-group
                        # behind the mask mults (v_sb gates macro 1's g2)
                        for c in range(ss * 4, ss * 4 + 4):
                            nc.vector.tensor_scalar(
                                v_sb[:, c, 0:KDIM], sub_sb[:, c, :],
                                w_sb[:, c : c + 1], None, OP.mult
                            )
                    if mac > 0:
                        for k in range(ss * per, (ss + 1) * per):
                            t, c = divmod(k, NKC)
                            g2_piece(mac - 1, pT_prev, t, c, g2_state)
                    if mac == NMAC - 1 and ss >= NSS // 2:
                        for t in range(TPM):
                            for c in own_sched[ss]:
                                g2_piece(mac, pT, t, c, g2_state_own, own=True)

                pT_prev = pT

        if loop_cm is not None:
            with loop_cm:
                body()
        else:
            body()

    nc.compile()
    return nc


def prep_in_maps(inputs):
    """Host-side dtype casts + layout packing + row-sharding (no model math):
    returns the 8 per-core input dicts for _build()'s dram tensors."""
    g = np.asarray(inputs["global_embeddings"]).astype(NP_BF16)      # [NQ, 512]
    sub = np.asarray(inputs["substruct_embeddings"]).astype(NP_BF16)  # [NK, 256]
    w = np.asarray(inputs["substruct_weight"], dtype=np.float32)
    mask = np.asarray(inputs["mask"])
    Wq = np.asarray(inputs["Wq"]).astype(NP_BF16)
    bq = np.asarray(inputs["bq"], dtype=np.float32)
    Wk = np.asarray(inputs["Wk"]).astype(NP_BF16)
    bk = np.asarray(inputs["bk"], dtype=np.float32)

    # shared (replicated) packs
    subQ = np.ascontiguousarray(
        sub.reshape(NKC, 128, KDIM).transpose(1, 0, 2))              # [128, NKC, KDIM]
    subTQ = np.ascontiguousarray(
        sub.T.reshape(2, 128, NK).transpose(1, 0, 2))                # [128, 2, NK]
    wQ = np.ascontiguousarray(w.reshape(32, 128))
    wqQ = np.ascontiguousarray(Wq.reshape(2, 128, QDIM).transpose(1, 0, 2))
    bqT = np.ascontiguousarray(bq.reshape(2, 128).T)
    wkQ = np.ascontiguousarray(Wk.reshape(2, 128, KDIM).transpose(1, 0, 2))
    bkT = np.ascontiguousarray(bk.reshape(2, 128).T)

    # maskQ[mac, p, c, q] = mask[core*R + mac*QMAC + q, c*128 + p]  (bf16 0/1)
    maskT = mask.T.astype(NP_BF16)                                    # [NK, NQ]
    gT = g.T                                                          # [512, NQ]

    in_maps = []
    for i in range(N_CORES):
        sl = slice(i * R, (i + 1) * R)
        mTc = maskT[:, sl]                                            # [NK, R]
        maskQ = np.ascontiguousarray(
            mTc.reshape(NKC, 128, NMAC, QMAC).transpose(2, 1, 0, 3))  # [NMAC,128,NKC,QMAC]
        gTc = gT[:, sl]                                               # [512, R]
        gQ = np.ascontiguousarray(
            gTc.reshape(4, 128, NMAC, QMAC).transpose(2, 1, 0, 3))    # [NMAC,128,4,QMAC]
        in_maps.append({
            "gQ": gQ, "maskQ": maskQ,
            "subQ": subQ, "subTQ": subTQ, "wQ": wQ,
            "wqQ": wqQ, "bqT": bqT, "wkQ": wkQ, "bkT": bkT,
        })
    return in_maps


_CACHE = {}


def kernel(**inputs) -> np.ndarray:
    """Full-input entry point: shards NQ across 8 NeuronCores, runs the Bass
    kernel, and gathers the full [16384, 256] float32 output."""
    if "nc" not in _CACHE:
        _CACHE["nc"] = _build()
    nc = _CACHE["nc"]

    in_maps = prep_in_maps(inputs)
    res = run_bass_kernel_spmd(nc, in_maps, list(range(N_CORES))).results
    return np.concatenate(
        [res[i]["out"] for i in range(N_CORES)], axis=0
    ).astype(np.float32)
